# revision 4
# baseline (speedup 1.0000x reference)
"""Trainium2 Bass kernel v2 for the AttentionRNNModel problem.

Math (fp32 reference):
    xi  = x @ W_i2h.T + b_i2h                      # [B,T,H]
    h_t = tanh(xi_t + h_{t-1} @ W_h2h.T + b_h2h)   # 512 sequential steps
    out = concat_t(h_t) @ W_fc.T + b_fc            # [B, O]

v2 changes over the 1.378ms baseline (all data-parallel over batch, 16
rows/core, h kept transposed [H=8x128 partitions, 16]):

1. W_h2h stored as fp8 e3m4 scaled by 2^8 (tanh un-scales via the
   activation's scale parameter). W_fc stays bf16: fp8 FC quantization
   noise scales with sqrt(N) exactly like the signal (~1.4% output err).
   Measured pair rate is nearly dtype-independent (~29.5ns at N=16);
   e3m4 buys only ~0.6ns/pair.
2. xi precomputed in 8-step blocks: one matmul per m-slab per block
   (N=128) instead of 8 per-step xb matmuls (N=16) — the bias rides in
   via the ones-row (K=65).  Blocks live in 2 PSUM banks (slabs 0-3 /
   4-7), double-buffered; the per-step W matmuls accumulate into them.
3. FC packed 5 steps per weight tile: wfc for steps 5C..5C+4 occupy
   disjoint 24-col blocks of one [128,128] stationary tile; one N=80
   matmul per (block, k) replaces 5 N=16 pairs.  The [120,80] psum
   accumulator's diagonal blocks are summed on the host.
4. h stored in a 20-slot SBUF ring so 5 consecutive steps form one
   strided moving AP for the FC matmuls.
5. Step emission order respects PSUM bank read/write exclusivity:
   bank-A matmuls first (its tanh fired mid-previous-step), bank-B
   matmuls and fillers later, covering tanh+semaphore latency.
"""

import numpy as np
import ml_dtypes

import concourse.bass as bass
import concourse.tile as tile
from concourse import bacc, mybir
from concourse.bass_utils import run_bass_kernel_spmd

B, T, D, H, O = 128, 512, 64, 1024, 24
NCORES = 8
BC = B // NCORES          # batch per core = 16
KM = H // 128             # 8 k-tiles / m-slabs
BF16 = mybir.dt.bfloat16
F32 = mybir.dt.float32

# --- precision config ---
W_DT = mybir.dt.float8e3      # e3m4: ~0.6ns/pair faster, err 0.0061 (gate 0.02)
FC_DT = mybir.dt.bfloat16     # W_fc must stay bf16: fp8 FC costs ~1.4% output err
SCALE_W = 256.0               # |W_h2h|<=1/32 -> x256 <= 8 (TRN e3m4 max 15.5)
SCALE_FC = 1.0

ABL_NO_FC = False             # timing ablation: skip all FC matmuls + DMAs
XI_BLK = 8                    # steps per xi-precompute block
FC_PACK = 5                   # steps per FC stationary tile (24*5=120<=128)
RING = 20                     # h ring slots (multiple of FC_PACK, >= 2*FC_PACK+2)
WFC_BUFS = 4

_np_dt = {mybir.dt.float8e3: ml_dtypes.float8_e3m4,
          mybir.dt.float8e4: ml_dtypes.float8_e4m3,
          mybir.dt.bfloat16: ml_dtypes.bfloat16}


def _build_program(t_steps: int, reps: int = 1):
    assert t_steps % XI_BLK == 0
    nblk_fc = (t_steps + FC_PACK - 1) // FC_PACK
    nc = bacc.Bacc("TRN2", target_bir_lowering=False, debug=False)

    wT_d = nc.dram_tensor("wT", [128, KM, H], W_DT, kind="ExternalInput")
    wiT_d = nc.dram_tensor("wiT", [D + 1, KM, 128], BF16, kind="ExternalInput")
    xTa_d = nc.dram_tensor("xTa", [D + 1, t_steps, BC], BF16, kind="ExternalInput")
    wfc_d = nc.dram_tensor("wfc", [nblk_fc, 128, KM, 128], FC_DT,
                           kind="ExternalInput")
    out_d = nc.dram_tensor("out", [120, FC_PACK * BC], F32, kind="ExternalOutput")

    with tile.TileContext(nc) as tc:
        with (
            tc.tile_pool(name="const", bufs=1) as const_pool,
            tc.tile_pool(name="wfc", bufs=WFC_BUFS) as wfc_pool,
            tc.tile_pool(name="xi", bufs=2, space=bass.MemorySpace.PSUM) as xi_pool,
            tc.tile_pool(name="fcps", bufs=1, space=bass.MemorySpace.PSUM) as fcps_pool,
            tc.tile_pool(name="outp", bufs=1) as out_pool,
        ):
            # ---- constants ----
            wiT = const_pool.tile([D + 1, KM, 128], BF16, tag="wiT", name="wiT")
            nc.sync.dma_start(wiT[:], wiT_d[:])
            xTa = const_pool.tile([D + 1, t_steps, BC], BF16, tag="xTa", name="xTa")
            wT = const_pool.tile([128, KM, H], W_DT, tag="wT", name="wT")
            nxc = 4
            xc = t_steps // nxc
            nc.gpsimd.dma_start(xTa[:, 0:xc, :], xTa_d[:, 0:xc, :])
            for k in range(KM):
                eng = nc.sync if k % 2 == 0 else nc.gpsimd
                eng.dma_start(wT[:, k, :], wT_d[:, k, :])
            for c in range(1, nxc):
                nc.gpsimd.dma_start(xTa[:, c * xc:(c + 1) * xc, :],
                                    xTa_d[:, c * xc:(c + 1) * xc, :])

            h_ring = const_pool.tile([128, RING, KM, BC], BF16, tag="hring",
                                     name="hring")
            fc_ps = fcps_pool.tile([128, FC_PACK * BC], F32, name="fcps")

            import contextlib
            rep_ctx = tc.For_i(0, reps) if reps > 1 else contextlib.nullcontext()
            with rep_ctx:
                _emit_body(nc, tc, t_steps, nblk_fc, wT, wiT, xTa, h_ring,
                           fc_ps, xi_pool, wfc_pool, out_pool, wfc_d, out_d)

    nc.compile()
    return nc


def _emit_xi_block(nc, xi_pool, wiT, xTa, blk, m, tiles):
    """Emit the xi matmul for slab m of xi block `blk` into its PSUM bank.

    tiles: dict blk -> (tileA, tileB); created on first use (m==0).
    """
    if m == 0:
        a = xi_pool.tile([128, 4, XI_BLK, BC], F32, tag="xiA", name="xiA")
        b = xi_pool.tile([128, 4, XI_BLK, BC], F32, tag="xiB", name="xiB")
        tiles[blk] = (a, b)
    a, b = tiles[blk]
    dst = a if m < 4 else b
    t0 = blk * XI_BLK
    nc.tensor.matmul(
        dst[:, m % 4, :, :], wiT[:, m, :], xTa[:, t0:t0 + XI_BLK, :],
        start=(m % 4 == 0), stop=False, skip_group_check=True,
    )


def _emit_fc_mm(nc, fc_ps, h_ring, wfc_tiles, t_steps, item):
    """One FC matmul: (block C, k-tile). Packs FC_PACK steps as weight cols."""
    if ABL_NO_FC:
        return
    C, k, is_last = item
    wfc_t = wfc_tiles[C]
    t0 = C * FC_PACK
    span = min(FC_PACK, t_steps - t0)
    s0 = t0 % RING
    rhs = h_ring[:, s0:s0 + span, k, :]
    if span == FC_PACK:
        nc.tensor.matmul(
            fc_ps[:], wfc_t[:, k, :], rhs,
            start=(C == 0 and k == 0), stop=is_last, skip_group_check=True,
        )
    else:  # tail block: span*24 weight cols, span*BC moving cols
        nc.tensor.matmul(
            fc_ps[0:span * O, 0:span * BC], wfc_t[:, k, 0:span * O], rhs,
            start=False, stop=is_last, skip_group_check=True,
        )


def _emit_body(nc, tc, t_steps, nblk_fc, wT, wiT, xTa, h_ring, fc_ps,
               xi_pool, wfc_pool, out_pool, wfc_d, out_d):
    nblk_xi = t_steps // XI_BLK
    xi_tiles = {}
    wfc_tiles = {}
    fc_queue = []
    pushed_fc = 0

    if ABL_NO_FC:
        nc.vector.memset(fc_ps[:], 0.0)
    # prologue: xi block 0, prefetch first FC weight blocks
    for m in range(KM):
        _emit_xi_block(nc, xi_pool, wiT, xTa, 0, m, xi_tiles)
    for C in range(min(3, nblk_fc)):
        wfc_tiles[C] = wfc_pool.tile([128, KM, 128], FC_DT, name="wfct")
        if not ABL_NO_FC:
            nc.sync.dma_start(wfc_tiles[C][:], wfc_d[C])

    inv_w = 1.0 / SCALE_W
    for t in range(t_steps):
        blk, i = divmod(t, XI_BLK)
        xiA, xiB = xi_tiles[blk]

        if t > 0:
            hp = (t - 1) % RING
            # phase 1a: bank-A slabs, k 0..3 — the only MMs whose read
            # (tanh A of t-1, fired mid-step) AND write bank (A) are both
            # comfortably old. Everything else waits behind these.
            for m in range(4):
                for k in range(4):
                    nc.tensor.matmul(
                        xiA[:, m, i, :],
                        wT[:, k, m * 128:(m + 1) * 128],
                        h_ring[:, hp, k, :],
                        start=False, stop=False, skip_group_check=True,
                    )
            # fillers: h-independent xi matmuls for the next block (2/step
            # from step 2 of the block) + FC matmuls for settled blocks.
            if blk + 1 < nblk_xi and i == 2:
                for m in range(KM):
                    _emit_xi_block(nc, xi_pool, wiT, xTa, blk + 1, m, xi_tiles)
            if len(fc_queue) >= 8 or (fc_queue and t >= t_steps - 2):
                for _ in range(8):
                    if fc_queue:
                        _emit_fc_mm(nc, fc_ps, h_ring, wfc_tiles, t_steps,
                                    fc_queue.pop(0))
            # phase 1b: bank-B slabs, k 0..3 (bank-B WAR on tanh B of t-1)
            for m in range(4, KM):
                for k in range(4):
                    nc.tensor.matmul(
                        xiB[:, m % 4, i, :],
                        wT[:, k, m * 128:(m + 1) * 128],
                        h_ring[:, hp, k, :],
                        start=False, stop=False, skip_group_check=True,
                    )
            # phase 2: k 4..7; finish bank A, fire its tanh, then bank B
            for m in range(4):
                for k in range(4, KM):
                    nc.tensor.matmul(
                        xiA[:, m, i, :],
                        wT[:, k, m * 128:(m + 1) * 128],
                        h_ring[:, hp, k, :],
                        start=False, stop=(k == KM - 1), skip_group_check=True,
                    )
            nc.scalar.activation(
                h_ring[:, t % RING, 0:4, :], xiA[:, :, i, :],
                mybir.ActivationFunctionType.Tanh, scale=inv_w,
            )
            for m in range(4, KM):
                for k in range(4, KM):
                    nc.tensor.matmul(
                        xiB[:, m % 4, i, :],
                        wT[:, k, m * 128:(m + 1) * 128],
                        h_ring[:, hp, k, :],
                        start=False, stop=(k == KM - 1), skip_group_check=True,
                    )
            nc.scalar.activation(
                h_ring[:, t % RING, 4:8, :], xiB[:, :, i, :],
                mybir.ActivationFunctionType.Tanh, scale=inv_w,
            )
        else:
            nc.scalar.activation(
                h_ring[:, t % RING, 0:4, :], xiA[:, :, i, :],
                mybir.ActivationFunctionType.Tanh, scale=inv_w,
            )
            nc.scalar.activation(
                h_ring[:, t % RING, 4:8, :], xiB[:, :, i, :],
                mybir.ActivationFunctionType.Tanh, scale=inv_w,
            )

        C = None
        if t % FC_PACK == 0 and t > 0:
            C = (t - 1) // FC_PACK
        if t == t_steps - 1:
            C = nblk_fc - 1  # last block(s): push everything still pending
        if C is not None:
            while pushed_fc <= C:
                Cp = pushed_fc
                last = (Cp == nblk_fc - 1)
                for k in range(KM):
                    fc_queue.append((Cp, k, last and k == KM - 1))
                pushed_fc += 1
            if C + 3 < nblk_fc:
                Cn = C + 3
                wfc_tiles[Cn] = wfc_pool.tile([128, KM, 128], FC_DT, name="wfct")
                if not ABL_NO_FC:
                    nc.sync.dma_start(wfc_tiles[Cn][:], wfc_d[Cn])
            wfc_tiles.pop(C - WFC_BUFS, None)

    while fc_queue:
        _emit_fc_mm(nc, fc_ps, h_ring, wfc_tiles, t_steps, fc_queue.pop(0))

    out_sb = out_pool.tile([120, FC_PACK * BC], F32, name="outsb")
    nc.vector.tensor_copy(out_sb[:], fc_ps[0:120, :])
    nc.sync.dma_start(out_d[:], out_sb[:])


def _prep_inputs(x, W_i2h, b_i2h, W_h2h, b_h2h, W_fc, t_steps):
    w_np = _np_dt[W_DT]
    fc_np = _np_dt[FC_DT]
    b_total = (np.asarray(b_i2h) + np.asarray(b_h2h)).astype(np.float32)

    # wT[p, kb, c] = W_h2h[c, kb*128+p] * SCALE_W
    wT = np.ascontiguousarray(
        np.asarray(W_h2h).T.reshape(KM, 128, H).transpose(1, 0, 2)
    ) * SCALE_W
    wT = wT.astype(w_np)

    # wiT[p<64, m, j] = W_i2h[m*128+j, p] * SCALE_W; row 64 = b_total * SCALE_W
    wiT = np.empty((D + 1, KM, 128), np.float32)
    wiT[:D] = np.asarray(W_i2h).T.reshape(D, KM, 128)
    wiT[D] = b_total.reshape(KM, 128)
    wiT = (wiT * SCALE_W).astype(ml_dtypes.bfloat16)

    # wfc packed: block C col 24*i+o = W_fc[o, (C*5+i)*1024 + k*128 + p]
    nblk_fc = (t_steps + FC_PACK - 1) // FC_PACK
    wfc_core = np.asarray(W_fc)[:, :t_steps * H] \
        .reshape(O, t_steps, KM, 128).transpose(1, 3, 2, 0)  # [t, p, k, o]
    wfcp = np.zeros((nblk_fc, 128, KM, 128), np.float32)
    for C in range(nblk_fc):
        t0 = C * FC_PACK
        span = min(FC_PACK, t_steps - t0)
        for i in range(span):
            wfcp[C, :, :, O * i:O * (i + 1)] = wfc_core[t0 + i]
    wfcp = (wfcp * SCALE_FC).astype(fc_np)

    # per-core xTa[p<64, t, b] = x[c*BC+b, t, p]; xTa[64] = 1.0
    xT = np.asarray(x)[:, :t_steps, :].transpose(2, 1, 0)  # [D, T, B]
    xTas = []
    for c in range(NCORES):
        xa = np.empty((D + 1, t_steps, BC), np.float32)
        xa[:D] = xT[:, :, c * BC:(c + 1) * BC]
        xa[D] = 1.0
        xTas.append(xa.astype(ml_dtypes.bfloat16))
    return wT, wiT, wfcp, xTas


def _extract_out(raw, t_steps):
    """raw: [120, 80] per-core psum dump -> [BC, O] fp32."""
    npack = min(FC_PACK, (t_steps + FC_PACK - 1))
    acc = np.zeros((O, BC), np.float32)
    for i in range(FC_PACK):
        acc += raw[O * i:O * (i + 1), BC * i:BC * (i + 1)]
    return acc.T / SCALE_FC


def _run(x, W_i2h, b_i2h, W_h2h, b_h2h, W_fc, b_fc, t_steps=T, trace=False):
    wT, wiT, wfcp, xTas = _prep_inputs(x, W_i2h, b_i2h, W_h2h, b_h2h, W_fc,
                                       t_steps)
    nc = _build_program(t_steps)
    in_maps = [
        {"wT": wT, "wiT": wiT, "xTa": xTas[c], "wfc": wfcp}
        for c in range(NCORES)
    ]
    res = run_bass_kernel_spmd(
        nc, in_maps, core_ids=list(range(NCORES)), trace=trace,
        **({"trace_cores": list(range(NCORES))} if trace else {}),
    )
    out = np.empty((B, O), np.float32)
    for c in range(NCORES):
        out[c * BC:(c + 1) * BC, :] = _extract_out(res.results[c]["out"], t_steps)
    out += np.asarray(b_fc, np.float32)[None, :]
    return out, res


def kernel(x, batchSize, W_i2h, b_i2h, W_h2h, b_h2h, W_fc, b_fc):
    out, _ = _run(x, W_i2h, b_i2h, W_h2h, b_h2h, W_fc, b_fc)
    return out

# Measured on the 8-core axon TRN2: 1.095 ms HW exec (2138 ns/step), rel
# err 6.1e-3 (vs 1.344 ms / 3.1e-3 baseline). Step budget: 64 W pairs x
# ~29.5ns (N=16 LDW+MM floor, dtype-independent) + xi-block amortized
# ~86ns + FC-pack ~106ns + residual scheduling overhead. Key measured
# facts: per-pair cost ~= N/2.4GHz + 28ns regardless of weight dtype;
# DoubleRow at FD=16 is 2x slower (121ns/instr); per-step collectives
# are infeasible (~us-scale latency floors), so batch-DP is optimal;
# bursting filler matmuls (FC/xi) into single insertion points saved
# ~90ns/step vs spreading them.
